# revision 31
# baseline (speedup 1.0000x reference)
"""Trainium2 Bass kernel for causal MHA (B=4, L=2048, D=1024, H=16), 8 cores.

Sharding: data-parallel over batch (4) x tensor-parallel over heads (2).
Each core handles one batch element and 8 heads (4 pairs):
  - QKV projection in bf16 (transposed-activation layout: [channel, token])
  - causal attention with exp-softmax (no max subtraction; inputs are small
    by construction so exp never overflows)
  - output projection partial sum over this core's 512 head-dims
Host pre-transposes x and the weights (layout prep), then sums the two
partial outputs of each batch pair.

Performance structure: every matmul input is bf16 (1 cycle/row on the PE;
fp32 runs 2-3x slower and the PE needs ~3us of continuous work to hold its
fast p-state, so the whole kernel is built to keep the PE queue dense).
Key pieces:
  - V stays resident in SBUF (no DRAM bounce); all x-chunks and qk tiles
    resident too.
  - exp is batched across the two heads of a pair: one ACTIVATE per
    key-block over a 2-bank PSUM tile ([P, 2, 512]).
  - score matmuls run one key-block ahead of the AV matmuls (software
    stagger) so AV never parks the PE on the exp that feeds it.
  - QK-projection chains for later pairs and output-projection chains are
    split into 2-matmul slices and paced into the attention emission as PE
    filler, closing the per-group gap between PE work and exp latency.
  - softmax normalization for token-chunk c is deferred into chunk c+1
    (denominator rows -> DMA transpose -> reciprocal -> DMA back -> one
    K=2 selector-matmul broadcast) so its round trip never stalls the PE;
    the final chunk uses a fast path with direct reciprocals and K=1
    broadcasts, with the last out-projection chains' independent halves
    emitted around it.
  - the causal tri-mask multiply runs on DVE against a stride-0 broadcast
    AP; the odd-head PSUM drain copy runs on the Activation engine
    (GpSimd cannot touch PSUM).
"""

import collections
import contextlib

import numpy as np

import concourse.bass as bass
import concourse.bacc as bacc
import concourse.mybir as mybir
import concourse.tile as tile

P = 128
HD = 64  # head dim

F32 = mybir.dt.float32
BF16 = mybir.dt.bfloat16


def build_mha_nc(L, D, HEADS):
    """Build the per-core Bass program (HEADS = heads per core)."""
    DBLK = D // P          # contraction blocks for projections
    KB = L // P            # key blocks
    MC = L // 512          # token chunks for projections
    EQK = 2 * HEADS * HD   # q+k output channels per core
    ET = EQK // P          # qk e-tiles (q/k pair-interleaved)
    EV = HEADS * HD        # v output channels per core
    PAIRS = HEADS // 2
    QS = min(512, L)       # q-span per AV-psum accumulation
    NQ = L // QS
    RPH = QS // P          # denominator rows per (chunk, head)
    assert L % 512 == 0 and D % P == 0 and EV % P == 0 and HEADS % 2 == 0

    nc = bacc.Bacc("TRN2", target_bir_lowering=False, debug=False,
                   enable_asserts=False)

    xT = nc.dram_tensor("xT", [D, L], BF16, kind="ExternalInput").ap()
    wT = nc.dram_tensor("wT", [D, EQK + EV], BF16, kind="ExternalInput").ap()
    bqk = nc.dram_tensor("bqk", [P, ET], F32, kind="ExternalInput").ap()
    vb = nc.dram_tensor("vb", [P, EV], F32, kind="ExternalInput").ap()
    woT = nc.dram_tensor("woT", [EV, D], BF16, kind="ExternalInput").ap()
    ob = nc.dram_tensor("ob", [P, D], F32, kind="ExternalInput").ap()
    tri = nc.dram_tensor("tri", [P, P], BF16, kind="ExternalInput").ap()
    onec = nc.dram_tensor("onec", [P, KB], BF16, kind="ExternalInput").ap()
    sel2 = nc.dram_tensor("sel2", [2, P], BF16, kind="ExternalInput").ap()
    onep = nc.dram_tensor("onep", [P, HD], BF16, kind="ExternalInput").ap()
    out = nc.dram_tensor("out", [L, D], F32, kind="ExternalOutput").ap()

    scale = 1.0 / float(np.sqrt(HD))

    with tile.TileContext(nc) as tc:
        ctx = contextlib.ExitStack()
        with ctx:
            consts = ctx.enter_context(tc.tile_pool(name="consts", bufs=1))
            wqk_pool = ctx.enter_context(tc.tile_pool(name="wqk", bufs=1))
            wvo_pool = ctx.enter_context(tc.tile_pool(name="wvo", bufs=1))
            xt_pool = ctx.enter_context(tc.tile_pool(name="xt", bufs=MC))
            qk_pool = ctx.enter_context(tc.tile_pool(name="qk", bufs=ET))
            vst_pool = ctx.enter_context(tc.tile_pool(name="vst", bufs=KB))
            ex_pool = ctx.enter_context(tc.tile_pool(name="ex", bufs=6))
            attn_pool = ctx.enter_context(tc.tile_pool(name="attn", bufs=1))
            outst_pool = ctx.enter_context(tc.tile_pool(name="outst", bufs=2))
            den_pool = ctx.enter_context(tc.tile_pool(name="den", bufs=2))
            recl_pool = ctx.enter_context(tc.tile_pool(name="recl", bufs=2))
            drow_pool = ctx.enter_context(tc.tile_pool(name="drow", bufs=2))
            tmp_pool = ctx.enter_context(tc.tile_pool(name="tmp", bufs=3))
            st_ps = ctx.enter_context(
                tc.tile_pool(name="st_ps", bufs=2, space="PSUM"))
            av_ps = ctx.enter_context(
                tc.tile_pool(name="av_ps", bufs=2, space="PSUM"))
            mm_ps = ctx.enter_context(
                tc.tile_pool(name="mm_ps", bufs=2, space="PSUM"))

            # ---- wv + x chunks first: they gate the first PE chains.
            # Big per-call transfers, spread across the three DMA-capable
            # queues; deferrable fp32 consts (ob/vb) come later ----
            wv_sb = wvo_pool.tile([P, DBLK, EV], BF16, name="wv_sb", tag="wvo")
            _wv_src = wT[:, EQK:EQK + EV].rearrange("(o p) e -> p o e", p=P)
            for _o in range(0, DBLK, DBLK // 2):
                nc.sync.dma_start(out=wv_sb[:, _o:_o + DBLK // 2, :],
                                  in_=_wv_src[:, _o:_o + DBLK // 2, :])

            xT_blocked = xT.rearrange("(o p) m -> p o m", p=P)
            xt_tiles = [xt_pool.tile([P, DBLK, 512], BF16, name=f"xt_{mc}",
                                     tag="xt") for mc in range(MC)]
            dma_engs = [nc.scalar, nc.gpsimd, nc.sync]
            di = 0
            for mc in range(MC):
                for _o in range(0, DBLK, DBLK // 2):
                    dma_engs[di % len(dma_engs)].dma_start(
                        out=xt_tiles[mc][:, _o:_o + DBLK // 2, :],
                        in_=xT_blocked[:, _o:_o + DBLK // 2,
                                       mc * 512:(mc + 1) * 512])
                    di += 1

            tri_sb = consts.tile([P, P], BF16, name="tri_sb")
            nc.gpsimd.dma_start(out=tri_sb, in_=tri)
            bqk_sb = consts.tile([P, ET], F32, name="bqk_sb")
            nc.gpsimd.dma_start(out=bqk_sb, in_=bqk)
            sel2_sb = consts.tile([2, P], BF16, name="sel2_sb")
            nc.gpsimd.dma_start(out=sel2_sb, in_=sel2)
            onep_sb = consts.tile([P, HD], BF16, name="onep_sb")
            nc.gpsimd.dma_start(out=onep_sb, in_=onep)
            ones_c = consts.tile([P, KB], BF16, name="ones_c")
            nc.gpsimd.dma_start(out=ones_c, in_=onec)
            tri_bc = tri_sb.unsqueeze(1).broadcast_to((P, 2, P))

            # ---- remaining weights (needed a few chains later) ----
            wqk_sb = wqk_pool.tile([P, DBLK, EQK], BF16, name="wqk_sb")
            _wqk_src = wT[:, 0:EQK].rearrange("(o p) e -> p o e", p=P)
            for _o in range(0, DBLK, 2):
                eng = nc.scalar if _o % 4 == 0 else nc.sync
                eng.dma_start(out=wqk_sb[:, _o:_o + 2, :],
                              in_=_wqk_src[:, _o:_o + 2, :])

            vb_sb = consts.tile([P, EV], F32, name="vb_sb")
            nc.gpsimd.dma_start(out=vb_sb, in_=vb)
            ob_sb = consts.tile([P, D], F32, name="ob_sb")
            nc.gpsimd.dma_start(out=ob_sb, in_=ob)

            # ---- V projection (resident in SBUF, + ones column) ----
            vb_h = vb_sb.rearrange("p (h c) -> p h c", c=HD)
            vst_tiles = []
            for kb in range(KB):
                mc, mt = kb // 4, kb % 4
                ps = mm_ps.tile([P, 512], F32, name=f"vps_{kb}", tag="mm")
                for o in range(DBLK):
                    nc.tensor.matmul(
                        ps[:, 0:EV],
                        lhsT=xt_tiles[mc][:, o, mt * P:(mt + 1) * P],
                        rhs=wv_sb[:, o, :],
                        start=(o == 0), stop=(o == DBLK - 1))
                vst = vst_pool.tile([P, HEADS, HD + 1], BF16,
                                    name=f"vst_{kb}", tag="vst")
                nc.vector.tensor_add(
                    out=vst[:, :, 0:HD],
                    in0=ps[:, 0:EV].rearrange("p (h c) -> p h c", c=HD),
                    in1=vb_h)
                nc.vector.tensor_copy(out=vst[:, :, HD:HD + 1],
                                      in_=ones_c[:, 0:HEADS, None])
                vst_tiles.append(vst)

            # wo loads reuse wv's SBUF space once V projection has read it
            wo_sb = wvo_pool.tile([P, EV // P, D], BF16, name="wo_sb",
                                  tag="wvo")
            _wo_src = woT.rearrange("(j p) f -> p j f", p=P)
            for _j in range(0, EV // P, 2):
                nc.sync.dma_start(out=wo_sb[:, _j:_j + 2, :],
                                  in_=_wo_src[:, _j:_j + 2, :])

            # ---- QK projection chains ----
            qk_tiles = [qk_pool.tile([P, L], BF16, name=f"qk_{et}", tag="qk")
                        for et in range(ET)]

            def qk_chain_parts(et, mc, nsplit=4):
                """Split one 8-matmul projection chain into nsplit filler
                slices so filler pacing stays fine-grained on the PE."""
                state = {}
                per = DBLK // nsplit
                def part(i):
                    def go():
                        if i == 0:
                            state["ps"] = mm_ps.tile(
                                [P, 512], F32, name=f"qkps_{et}_{mc}",
                                tag="mm")
                        ps = state["ps"]
                        for o in range(i * per, (i + 1) * per):
                            nc.tensor.matmul(
                                ps,
                                lhsT=wqk_sb[:, o, et * P:(et + 1) * P],
                                rhs=xt_tiles[mc][:, o, :],
                                start=(o == 0), stop=(o == DBLK - 1))
                        if i == nsplit - 1:
                            nc.vector.tensor_scalar(
                                out=qk_tiles[et][:, mc * 512:(mc + 1) * 512],
                                in0=ps, scalar1=bqk_sb[:, et:et + 1],
                                scalar2=None, op0=mybir.AluOpType.add)
                    return go
                return [part(i) for i in range(nsplit)]

            # e-tiles for pair 0 up front; the rest become attention filler
            for et in (0, 1):
                for mc in range(MC):
                    for p_ in qk_chain_parts(et, mc, nsplit=1):
                        p_()

            # ---- output projection chain (split into 2-matmul slices) ----
            def out_chain_parts(qt, f0, fn, nsplit=2, pool=None, ptag=None):
                state = {}
                NJ = EV // P
                per = NJ // nsplit
                def part(i):
                    def go():
                        if i == 0:
                            state["ps"] = (pool or mm_ps).tile(
                                [P, 512], F32, name=f"ops_{qt}_{f0}",
                                tag=ptag or "mm")
                        ps = state["ps"]
                        for j in range(i * per, (i + 1) * per):
                            nc.tensor.matmul(
                                ps[:, 0:fn],
                                lhsT=attn_sb[:, j, qt * P:(qt + 1) * P],
                                rhs=wo_sb[:, j, f0:f0 + fn],
                                start=(j == 0), stop=(j == NJ - 1))
                        if i == nsplit - 1:
                            ot = outst_pool.tile(
                                [P, 512], F32, name=f"ot_{qt}_{f0}",
                                tag="outst")
                            nc.vector.tensor_add(
                                out=ot[:, 0:fn], in0=ps[:, 0:fn],
                                in1=ob_sb[:, f0:f0 + fn])
                            nc.sync.dma_start(
                                out=out[qt * P:(qt + 1) * P, f0:f0 + fn],
                                in_=ot[:, 0:fn])
                    return go
                return [part(i) for i in range(nsplit)]

            # ---- PE filler pacing ----
            class Fillers:
                def __init__(self):
                    self.q = collections.deque()
                    self.emitted = 0
                    self.ticks = 0
                    self.rate = 0.0

                def push(self, fn):
                    self.q.append(fn)

                def tick(self):
                    self.ticks += 1
                    while self.q and self.emitted < self.rate * self.ticks:
                        self.q.popleft()()
                        self.emitted += 1

                def drain(self):
                    while self.q:
                        self.q.popleft()()

                def set_rate(self, rate):
                    self.rate = rate
                    self.emitted = 0
                    self.ticks = 0

            fillers = Fillers()
            for et in range(2, ET):
                for mc in range(MC):
                    for p_ in qk_chain_parts(et, mc):
                        fillers.push(p_)
            # 96 remaining qk chain-slices over pairs 0..2 (120 kb-groups);
            # out-proj slices unlock progressively during pair 3
            fillers.rate = len(fillers.q) / (
                2 * 3 * (NQ * (NQ + 1) // 2) * 4)

            attn_sb = attn_pool.tile([P, PAIRS, L], BF16, name="attn_sb")

            pending_norm = collections.deque()

            def attention_pair(pr, last_pair):
                q_tile = qk_tiles[2 * pr]
                k_tile = qk_tiles[2 * pr + 1]
                # chunk index on the free dim so each reciprocal slice sits
                # at base partition 0 (engine partition-base constraint)
                den = den_pool.tile([2 * RPH, NQ, P], BF16,
                                    name=f"den_{pr}", tag="den")
                recl = recl_pool.tile([2, NQ, QS], BF16,
                                      name=f"recl_{pr}", tag="recl")
                for q4 in range(NQ):
                    q0 = q4 * QS
                    avs = [av_ps.tile([HD + 1, QS], F32,
                                      name=f"av_{pr}_{q4}_{hh}", tag="av")
                           for hh in (0, 1)]
                    last_kb = (q0 + QS) // P - 1
                    pending_av = None
                    for kb in range(last_kb + 1):
                        s0 = max(0, kb * P - q0)
                        d0 = kb * P - q0
                        st = st_ps.tile([P, 2, QS], F32,
                                        name=f"st_{pr}_{q4}_{kb}", tag="st")
                        for hh in (0, 1):
                            rows = slice(hh * HD, hh * HD + HD)
                            nc.tensor.matmul(
                                st[:, hh, s0:QS],
                                lhsT=k_tile[rows, kb * P:(kb + 1) * P],
                                rhs=q_tile[rows, q0 + s0:q0 + QS],
                                start=True, stop=True)
                        ex = ex_pool.tile([P, 2, QS], BF16,
                                          name=f"ex_{pr}_{q4}_{kb}", tag="ex")
                        nc.scalar.activation(
                            out=ex[:, :, s0:QS], in_=st[:, :, s0:QS],
                            func=mybir.ActivationFunctionType.Exp,
                            scale=scale)
                        if d0 >= 0:
                            nc.vector.tensor_mul(
                                out=ex[:, :, d0:d0 + P],
                                in0=ex[:, :, d0:d0 + P], in1=tri_bc)
                        fillers.tick()
                        # AV for the previous kb-group: one group behind the
                        # score matmuls, so the PE never parks on exp(kb)
                        if pending_av is not None:
                            pending_av()
                        def make_av(kb=kb, s0=s0, ex=ex):
                            def go():
                                for hh in (0, 1):
                                    nc.tensor.matmul(
                                        avs[hh][:, s0:QS],
                                        lhsT=vst_tiles[kb][:, 2 * pr + hh, :],
                                        rhs=ex[:, hh, s0:QS],
                                        start=(kb == 0),
                                        stop=(kb == last_kb))
                            return go
                        pending_av = make_av()
                        pop_kb = 1 if last_pair else max(1, last_kb - 1)
                        if kb == pop_kb and pending_norm:
                            pending_norm.popleft()()
                        fillers.tick()
                    pending_av()
                    # drain av psum (GpSimd): raw attn rows + den rows
                    dr = drow_pool.tile([1, QS], BF16, name=f"dr_{pr}_{q4}",
                                        tag="drow")
                    tmp = tmp_pool.tile([HD + 1, QS], BF16,
                                        name=f"tmp_{pr}_{q4}", tag="tmp")
                    nc.vector.tensor_copy(
                        out=attn_sb[0:HD, pr, q0:q0 + QS], in_=avs[0][0:HD, :])
                    nc.vector.tensor_copy(out=dr, in_=avs[0][HD:HD + 1, :])
                    nc.scalar.copy(out=tmp, in_=avs[1])
                    fast_tail = last_pair and q4 == NQ - 1
                    if not fast_tail:
                        for hh, src in ((0, dr[0:1, :]),
                                        (1, tmp[HD:HD + 1, :])):
                            nc.sync.dma_start(
                                out=den[hh * RPH:(hh + 1) * RPH, q4, :],
                                in_=src)

                    def normalize(q4=q4, q0=q0, den=den, recl=recl, tmp=tmp,
                                  dr=dr, pr=pr, last_pair=last_pair,
                                  fast_tail=fast_tail):
                        if fast_tail:
                            # tail: reciprocal directly on the raw den rows
                            # (partitions 0 and 64) and two K=1 broadcasts —
                            # skips both DMA bounce trips
                            with nc.allow_low_precision(
                                    reason="bf16 softmax denominator"):
                                nc.vector.reciprocal(out=dr, in_=dr)
                                nc.vector.reciprocal(
                                    out=tmp[HD:HD + 1, :],
                                    in_=tmp[HD:HD + 1, :])
                            bps = av_ps.tile([P, QS], F32,
                                             name=f"bpst_{pr}_{q4}",
                                             tag="av")
                            nc.tensor.matmul(
                                bps[0:HD, :], lhsT=onep_sb[0:1, :], rhs=dr,
                                start=True, stop=True)
                            nc.tensor.matmul(
                                bps[HD:P, :], lhsT=onep_sb[HD:HD + 1, :],
                                rhs=tmp[HD:HD + 1, :],
                                start=True, stop=True)
                            sl = attn_sb[0:HD, pr, q0:q0 + QS]
                            nc.vector.tensor_mul(out=sl, in0=sl,
                                                 in1=bps[0:HD, :])
                            nc.vector.tensor_mul(out=tmp[0:HD, :],
                                                 in0=tmp[0:HD, :],
                                                 in1=bps[HD:P, :])
                            nc.sync.dma_start(
                                out=attn_sb[HD:P, pr, q0:q0 + QS],
                                in_=tmp[0:HD, :])
                            return
                        dsl = den[:, q4, :]
                        with nc.allow_low_precision(
                                reason="bf16 softmax denominator"):
                            nc.vector.reciprocal(out=dsl, in_=dsl)
                        for hh in (0, 1):
                            nc.sync.dma_start(
                                out=recl[hh:hh + 1, q4, :],
                                in_=den[hh * RPH:(hh + 1) * RPH, q4, :])
                        # one K=2 broadcast: rows 0:64 get head0's recip,
                        # rows 64:128 head1's (sel2 is the 0/1 selector)
                        bps = mm_ps.tile([P, QS], F32,
                                         name=f"bps_{pr}_{q4}", tag="mm")
                        nc.tensor.matmul(
                            bps, lhsT=sel2_sb, rhs=recl[:, q4, :],
                            start=True, stop=True)
                        sl = attn_sb[0:HD, pr, q0:q0 + QS]
                        nc.vector.tensor_mul(out=sl, in0=sl,
                                             in1=bps[0:HD, :])
                        nc.vector.tensor_mul(out=tmp[0:HD, :],
                                             in0=tmp[0:HD, :],
                                             in1=bps[HD:P, :])
                        nc.sync.dma_start(
                            out=attn_sb[HD:P, pr, q0:q0 + QS],
                            in_=tmp[0:HD, :])
                        if last_pair:
                            for qt in range(q4 * (QS // P),
                                            (q4 + 1) * (QS // P)):
                                for f0 in range(0, D, 512):
                                    for p_ in out_chain_parts(
                                            qt, f0, min(512, D - f0)):
                                        fillers.push(p_)
                            # pace the queue across the next chunk's ticks
                            nxt = 2 * (QS // P) * (q4 + 2)
                            fillers.set_rate(len(fillers.q) / nxt)

                    pending_norm.append(normalize)

            for pr in range(PAIRS):
                if pr == PAIRS - 1:
                    # switch pacing to out-proj chain slices (2/kb-group)
                    fillers.drain()
                    fillers.emitted = 0
                    fillers.ticks = 0
                    fillers.rate = 1.0
                attention_pair(pr, pr == PAIRS - 1)
            fillers.drain()
            # tail: the final chunk's out chains. Emit their j=0,1 halves
            # (pairs 0,1 — long since final) BEFORE the fast normalize so
            # they hide its latency; j=2,3 halves follow it.
            q4t = NQ - 1
            tail_specs = [(qt, f0)
                          for qt in range(q4t * (QS // P),
                                          (q4t + 1) * (QS // P))
                          for f0 in range(0, D, 512)]
            tail_parts = [
                out_chain_parts(qt, f0, min(512, D - f0),
                                pool=(st_ps if k % 2 else mm_ps),
                                ptag=("st" if k % 2 else "mm"))
                for k, (qt, f0) in enumerate(tail_specs)]
            for k in range(min(4, len(tail_parts))):
                tail_parts[k][0]()
            while pending_norm:
                pending_norm.popleft()()
            for k in range(len(tail_parts)):
                tail_parts[k][1]()
                if k + 4 < len(tail_parts):
                    tail_parts[k + 4][0]()

    nc.compile()
    return nc


def to_bf16(a):
    import ml_dtypes
    return np.ascontiguousarray(a).astype(ml_dtypes.bfloat16)


def make_core_inputs(x, Wqkv_w, Wqkv_b, out_w, out_b, H, n_tp):
    """Host-side shard + layout prep. Returns list of in_maps (one per core).
    Core c handles batch c // n_tp, head group c % n_tp."""
    B, L, D = x.shape
    hpg = H // n_tp            # heads per core
    PAIRS = hpg // 2
    EQK = 2 * hpg * HD
    EV = hpg * HD
    ET = EQK // P
    tri = np.triu(np.ones((P, P), dtype=np.float32))  # [k, q]: 1 if q >= k
    in_maps = []
    for c in range(B * n_tp):
        b, g = c // n_tp, c % n_tp
        # qk row order: per pair p -> q(2p), q(2p+1), k(2p), k(2p+1)
        qk_rows = []
        for p_ in range(PAIRS):
            for h in (2 * p_, 2 * p_ + 1):
                qk_rows.extend(range(g * hpg * HD + h * HD,
                                     g * hpg * HD + h * HD + HD))
            for h in (2 * p_, 2 * p_ + 1):
                qk_rows.extend(range(D + g * hpg * HD + h * HD,
                                     D + g * hpg * HD + h * HD + HD))
        v_rows = list(range(2 * D + g * hpg * HD, 2 * D + (g + 1) * hpg * HD))
        rows = np.array(qk_rows + v_rows)
        in_maps.append({
            "xT": to_bf16(x[b].T),
            "wT": to_bf16(Wqkv_w[rows].T),
            "bqk": np.ascontiguousarray(
                Wqkv_b[np.array(qk_rows)].reshape(ET, P).T),
            "vb": np.tile(Wqkv_b[np.array(v_rows)], (P, 1)),
            "woT": to_bf16(out_w[:, g * EV:(g + 1) * EV].T),
            "ob": (np.tile(out_b, (P, 1)) if g == 0
                   else np.zeros((P, D), np.float32)),
            "tri": to_bf16(tri),
            "onep": to_bf16(np.ones((P, HD), np.float32)),
            "sel2": to_bf16(np.concatenate([
                np.concatenate([np.ones((1, HD)), np.zeros((1, HD))], 1),
                np.concatenate([np.zeros((1, HD)), np.ones((1, HD))], 1),
            ]).astype(np.float32)),
            "onec": to_bf16(np.ones((P, L // P), np.float32)),
        })
    return in_maps


_NC_CACHE = {}
LAST_RESULTS = None


def kernel(x, Wqkv_w, Wqkv_b, out_w, out_b):
    global LAST_RESULTS
    x = np.asarray(x, dtype=np.float32)
    Wqkv_w = np.asarray(Wqkv_w, dtype=np.float32)
    Wqkv_b = np.asarray(Wqkv_b, dtype=np.float32)
    out_w = np.asarray(out_w, dtype=np.float32)
    out_b = np.asarray(out_b, dtype=np.float32)

    B, L, D = x.shape
    H = 16
    n_tp = 2
    hpg = H // n_tp

    key = (L, D, hpg)
    if key not in _NC_CACHE:
        _NC_CACHE[key] = build_mha_nc(L, D, hpg)
    nc = _NC_CACHE[key]

    in_maps = make_core_inputs(x, Wqkv_w, Wqkv_b, out_w, out_b, H, n_tp)

    from concourse.bass_utils import run_bass_kernel_spmd
    res = run_bass_kernel_spmd(nc, in_maps, core_ids=list(range(len(in_maps))))
    LAST_RESULTS = res

    out = np.empty((B, L, D), dtype=np.float32)
    for b in range(B):
        out[b] = res.results[n_tp * b]["out"]
        for g in range(1, n_tp):
            out[b] += res.results[n_tp * b + g]["out"]
    return out


if __name__ == "__main__":
    nc = build_mha_nc(2048, 1024, 8)
    print("built OK")


# revision 32
# speedup vs baseline: 1.0289x; 1.0289x over previous
"""Trainium2 Bass kernel for causal MHA (B=4, L=2048, D=1024, H=16), 8 cores.

Sharding: data-parallel over batch (4) x tensor-parallel over heads (2).
Each core handles one batch element and 8 heads (4 pairs):
  - QKV projection in bf16 (transposed-activation layout: [channel, token])
  - causal attention with exp-softmax (no max subtraction; inputs are small
    by construction so exp never overflows)
  - output projection partial sum over this core's 512 head-dims
Host pre-transposes x and the weights (layout prep), then sums the two
partial outputs of each batch pair.

Performance structure: every matmul input is bf16 (1 cycle/row on the PE;
fp32 runs 2-3x slower and the PE needs ~3us of continuous work to hold its
fast p-state, so the whole kernel is built to keep the PE queue dense).
Key pieces:
  - V stays resident in SBUF (no DRAM bounce); all x-chunks and qk tiles
    resident too.
  - exp is batched across the two heads of a pair: one ACTIVATE per
    key-block over a 2-bank PSUM tile ([P, 2, 512]).
  - score matmuls run one key-block ahead of the AV matmuls (software
    stagger) so AV never parks the PE on the exp that feeds it.
  - QK-projection chains for later pairs and output-projection chains are
    split into 2-matmul slices and paced into the attention emission as PE
    filler, closing the per-group gap between PE work and exp latency.
  - softmax normalization for token-chunk c is deferred into chunk c+1
    (denominator rows -> DMA transpose -> reciprocal -> DMA back -> one
    K=2 selector-matmul broadcast) so its round trip never stalls the PE;
    the final chunk uses a fast path with direct reciprocals and K=1
    broadcasts, with the last out-projection chains' independent halves
    emitted around it.
  - the causal tri-mask multiply runs on DVE against a stride-0 broadcast
    AP; the odd-head PSUM drain copy runs on the Activation engine
    (GpSimd cannot touch PSUM).
"""

import collections
import contextlib

import numpy as np

import concourse.bass as bass
import concourse.bacc as bacc
import concourse.mybir as mybir
import concourse.tile as tile

P = 128
HD = 64  # head dim

F32 = mybir.dt.float32
BF16 = mybir.dt.bfloat16


def build_mha_nc(L, D, HEADS):
    """Build the per-core Bass program (HEADS = heads per core)."""
    DBLK = D // P          # contraction blocks for projections
    KB = L // P            # key blocks
    MC = L // 512          # token chunks for projections
    EQK = 2 * HEADS * HD   # q+k output channels per core
    ET = EQK // P          # qk e-tiles (q/k pair-interleaved)
    EV = HEADS * HD        # v output channels per core
    PAIRS = HEADS // 2
    QS = min(512, L)       # q-span per AV-psum accumulation
    NQ = L // QS
    RPH = QS // P          # denominator rows per (chunk, head)
    assert L % 512 == 0 and D % P == 0 and EV % P == 0 and HEADS % 2 == 0

    nc = bacc.Bacc("TRN2", target_bir_lowering=False, debug=False,
                   enable_asserts=False)

    xT = nc.dram_tensor("xT", [D, L], BF16, kind="ExternalInput").ap()
    wT = nc.dram_tensor("wT", [D, EQK + EV], BF16, kind="ExternalInput").ap()
    bqk = nc.dram_tensor("bqk", [P, ET], F32, kind="ExternalInput").ap()
    vb = nc.dram_tensor("vb", [P, EV], F32, kind="ExternalInput").ap()
    woT = nc.dram_tensor("woT", [EV, D], BF16, kind="ExternalInput").ap()
    ob = nc.dram_tensor("ob", [P, D], F32, kind="ExternalInput").ap()
    tri = nc.dram_tensor("tri", [P, P], BF16, kind="ExternalInput").ap()
    onec = nc.dram_tensor("onec", [P, KB], BF16, kind="ExternalInput").ap()
    sel2 = nc.dram_tensor("sel2", [2, P], BF16, kind="ExternalInput").ap()
    onep = nc.dram_tensor("onep", [P, HD], BF16, kind="ExternalInput").ap()
    out = nc.dram_tensor("out", [L, D], F32, kind="ExternalOutput").ap()

    scale = 1.0 / float(np.sqrt(HD))

    with tile.TileContext(nc) as tc:
        ctx = contextlib.ExitStack()
        with ctx:
            consts = ctx.enter_context(tc.tile_pool(name="consts", bufs=1))
            wqk_pool = ctx.enter_context(tc.tile_pool(name="wqk", bufs=1))
            wvo_pool = ctx.enter_context(tc.tile_pool(name="wvo", bufs=1))
            xt_pool = ctx.enter_context(tc.tile_pool(name="xt", bufs=MC))
            qk_pool = ctx.enter_context(tc.tile_pool(name="qk", bufs=ET))
            vst_pool = ctx.enter_context(tc.tile_pool(name="vst", bufs=KB))
            ex_pool = ctx.enter_context(tc.tile_pool(name="ex", bufs=6))
            attn_pool = ctx.enter_context(tc.tile_pool(name="attn", bufs=1))
            outst_pool = ctx.enter_context(tc.tile_pool(name="outst", bufs=4))
            den_pool = ctx.enter_context(tc.tile_pool(name="den", bufs=2))
            recl_pool = ctx.enter_context(tc.tile_pool(name="recl", bufs=2))
            drow_pool = ctx.enter_context(tc.tile_pool(name="drow", bufs=2))
            tmp_pool = ctx.enter_context(tc.tile_pool(name="tmp", bufs=3))
            st_ps = ctx.enter_context(
                tc.tile_pool(name="st_ps", bufs=2, space="PSUM"))
            av_ps = ctx.enter_context(
                tc.tile_pool(name="av_ps", bufs=2, space="PSUM"))
            mm_ps = ctx.enter_context(
                tc.tile_pool(name="mm_ps", bufs=2, space="PSUM"))

            # ---- wv + x chunks first: they gate the first PE chains.
            # Big per-call transfers, spread across the three DMA-capable
            # queues; deferrable fp32 consts (ob/vb) come later ----
            wv_sb = wvo_pool.tile([P, DBLK, EV], BF16, name="wv_sb", tag="wvo")
            _wv_src = wT[:, EQK:EQK + EV].rearrange("(o p) e -> p o e", p=P)
            for _o in range(0, DBLK, DBLK // 2):
                nc.sync.dma_start(out=wv_sb[:, _o:_o + DBLK // 2, :],
                                  in_=_wv_src[:, _o:_o + DBLK // 2, :])

            xT_blocked = xT.rearrange("(o p) m -> p o m", p=P)
            xt_tiles = [xt_pool.tile([P, DBLK, 512], BF16, name=f"xt_{mc}",
                                     tag="xt") for mc in range(MC)]
            dma_engs = [nc.scalar, nc.gpsimd, nc.sync]
            di = 0
            for mc in range(MC):
                for _o in range(0, DBLK, DBLK // 2):
                    dma_engs[di % len(dma_engs)].dma_start(
                        out=xt_tiles[mc][:, _o:_o + DBLK // 2, :],
                        in_=xT_blocked[:, _o:_o + DBLK // 2,
                                       mc * 512:(mc + 1) * 512])
                    di += 1

            tri_sb = consts.tile([P, P], BF16, name="tri_sb")
            nc.gpsimd.dma_start(out=tri_sb, in_=tri)
            bqk_sb = consts.tile([P, ET], F32, name="bqk_sb")
            nc.gpsimd.dma_start(out=bqk_sb, in_=bqk)
            sel2_sb = consts.tile([2, P], BF16, name="sel2_sb")
            nc.gpsimd.dma_start(out=sel2_sb, in_=sel2)
            onep_sb = consts.tile([P, HD], BF16, name="onep_sb")
            nc.gpsimd.dma_start(out=onep_sb, in_=onep)
            ones_c = consts.tile([P, KB], BF16, name="ones_c")
            nc.gpsimd.dma_start(out=ones_c, in_=onec)
            tri_bc = tri_sb.unsqueeze(1).broadcast_to((P, 2, P))

            # ---- remaining weights (needed a few chains later) ----
            wqk_sb = wqk_pool.tile([P, DBLK, EQK], BF16, name="wqk_sb")
            _wqk_src = wT[:, 0:EQK].rearrange("(o p) e -> p o e", p=P)
            for _o in range(0, DBLK, 2):
                eng = nc.scalar if _o % 4 == 0 else nc.sync
                eng.dma_start(out=wqk_sb[:, _o:_o + 2, :],
                              in_=_wqk_src[:, _o:_o + 2, :])

            vb_sb = consts.tile([P, EV], F32, name="vb_sb")
            nc.gpsimd.dma_start(out=vb_sb, in_=vb)
            ob_sb = consts.tile([P, D], F32, name="ob_sb")
            nc.gpsimd.dma_start(out=ob_sb, in_=ob)

            # ---- V projection (resident in SBUF, + ones column) ----
            vb_h = vb_sb.rearrange("p (h c) -> p h c", c=HD)
            vst_tiles = []
            for kb in range(KB):
                mc, mt = kb // 4, kb % 4
                ps = mm_ps.tile([P, 512], F32, name=f"vps_{kb}", tag="mm")
                for o in range(DBLK):
                    nc.tensor.matmul(
                        ps[:, 0:EV],
                        lhsT=xt_tiles[mc][:, o, mt * P:(mt + 1) * P],
                        rhs=wv_sb[:, o, :],
                        start=(o == 0), stop=(o == DBLK - 1))
                vst = vst_pool.tile([P, HEADS, HD + 1], BF16,
                                    name=f"vst_{kb}", tag="vst")
                nc.vector.tensor_add(
                    out=vst[:, :, 0:HD],
                    in0=ps[:, 0:EV].rearrange("p (h c) -> p h c", c=HD),
                    in1=vb_h)
                nc.vector.tensor_copy(out=vst[:, :, HD:HD + 1],
                                      in_=ones_c[:, 0:HEADS, None])
                vst_tiles.append(vst)

            # wo loads reuse wv's SBUF space once V projection has read it
            wo_sb = wvo_pool.tile([P, EV // P, D], BF16, name="wo_sb",
                                  tag="wvo")
            _wo_src = woT.rearrange("(j p) f -> p j f", p=P)
            for _j in range(0, EV // P, 2):
                nc.sync.dma_start(out=wo_sb[:, _j:_j + 2, :],
                                  in_=_wo_src[:, _j:_j + 2, :])

            # ---- QK projection chains ----
            qk_tiles = [qk_pool.tile([P, L], BF16, name=f"qk_{et}", tag="qk")
                        for et in range(ET)]

            def qk_chain_parts(et, mc, nsplit=4):
                """Split one 8-matmul projection chain into nsplit filler
                slices so filler pacing stays fine-grained on the PE."""
                state = {}
                per = DBLK // nsplit
                def part(i):
                    def go():
                        if i == 0:
                            state["ps"] = mm_ps.tile(
                                [P, 512], F32, name=f"qkps_{et}_{mc}",
                                tag="mm")
                        ps = state["ps"]
                        for o in range(i * per, (i + 1) * per):
                            nc.tensor.matmul(
                                ps,
                                lhsT=wqk_sb[:, o, et * P:(et + 1) * P],
                                rhs=xt_tiles[mc][:, o, :],
                                start=(o == 0), stop=(o == DBLK - 1))
                        if i == nsplit - 1:
                            nc.vector.tensor_scalar(
                                out=qk_tiles[et][:, mc * 512:(mc + 1) * 512],
                                in0=ps, scalar1=bqk_sb[:, et:et + 1],
                                scalar2=None, op0=mybir.AluOpType.add)
                    return go
                return [part(i) for i in range(nsplit)]

            # e-tiles for pair 0 up front; the rest become attention filler
            for et in (0, 1):
                for mc in range(MC):
                    for p_ in qk_chain_parts(et, mc, nsplit=1):
                        p_()

            # ---- output projection chain (split into 2-matmul slices) ----
            def out_chain_parts(qt, f0, fn, nsplit=2, pool=None, ptag=None):
                state = {}
                NJ = EV // P
                per = NJ // nsplit
                def part(i):
                    def go():
                        if i == 0:
                            state["ps"] = (pool or mm_ps).tile(
                                [P, 512], F32, name=f"ops_{qt}_{f0}",
                                tag=ptag or "mm")
                        ps = state["ps"]
                        for j in range(i * per, (i + 1) * per):
                            nc.tensor.matmul(
                                ps[:, 0:fn],
                                lhsT=attn_sb[:, j, qt * P:(qt + 1) * P],
                                rhs=wo_sb[:, j, f0:f0 + fn],
                                start=(j == 0), stop=(j == NJ - 1))
                        if i == nsplit - 1:
                            ot = outst_pool.tile(
                                [P, 512], F32, name=f"ot_{qt}_{f0}",
                                tag="outst")
                            nc.vector.tensor_add(
                                out=ot[:, 0:fn], in0=ps[:, 0:fn],
                                in1=ob_sb[:, f0:f0 + fn])
                            nc.sync.dma_start(
                                out=out[qt * P:(qt + 1) * P, f0:f0 + fn],
                                in_=ot[:, 0:fn])
                    return go
                return [part(i) for i in range(nsplit)]

            # ---- PE filler pacing ----
            class Fillers:
                def __init__(self):
                    self.q = collections.deque()
                    self.emitted = 0
                    self.ticks = 0
                    self.rate = 0.0

                def push(self, fn):
                    self.q.append(fn)

                def tick(self):
                    self.ticks += 1
                    while self.q and self.emitted < self.rate * self.ticks:
                        self.q.popleft()()
                        self.emitted += 1

                def drain(self):
                    while self.q:
                        self.q.popleft()()

                def set_rate(self, rate):
                    self.rate = rate
                    self.emitted = 0
                    self.ticks = 0

            fillers = Fillers()
            for et in range(2, ET):
                for mc in range(MC):
                    for p_ in qk_chain_parts(et, mc):
                        fillers.push(p_)
            # 96 remaining qk chain-slices over pairs 0..2 (120 kb-groups);
            # out-proj slices unlock progressively during pair 3
            fillers.rate = len(fillers.q) / (
                2 * 3 * (NQ * (NQ + 1) // 2) * 4)

            attn_sb = attn_pool.tile([P, PAIRS, L], BF16, name="attn_sb")

            pending_norm = collections.deque()

            def attention_pair(pr, last_pair):
                q_tile = qk_tiles[2 * pr]
                k_tile = qk_tiles[2 * pr + 1]
                # chunk index on the free dim so each reciprocal slice sits
                # at base partition 0 (engine partition-base constraint)
                den = den_pool.tile([2 * RPH, NQ, P], BF16,
                                    name=f"den_{pr}", tag="den")
                recl = recl_pool.tile([2, NQ, QS], BF16,
                                      name=f"recl_{pr}", tag="recl")
                for q4 in range(NQ):
                    q0 = q4 * QS
                    avs = [av_ps.tile([HD + 1, QS], F32,
                                      name=f"av_{pr}_{q4}_{hh}", tag="av")
                           for hh in (0, 1)]
                    last_kb = (q0 + QS) // P - 1
                    pending_av = None
                    for kb in range(last_kb + 1):
                        s0 = max(0, kb * P - q0)
                        d0 = kb * P - q0
                        st = st_ps.tile([P, 2, QS], F32,
                                        name=f"st_{pr}_{q4}_{kb}", tag="st")
                        for hh in (0, 1):
                            rows = slice(hh * HD, hh * HD + HD)
                            nc.tensor.matmul(
                                st[:, hh, s0:QS],
                                lhsT=k_tile[rows, kb * P:(kb + 1) * P],
                                rhs=q_tile[rows, q0 + s0:q0 + QS],
                                start=True, stop=True)
                        ex = ex_pool.tile([P, 2, QS], BF16,
                                          name=f"ex_{pr}_{q4}_{kb}", tag="ex")
                        nc.scalar.activation(
                            out=ex[:, :, s0:QS], in_=st[:, :, s0:QS],
                            func=mybir.ActivationFunctionType.Exp,
                            scale=scale)
                        if d0 >= 0:
                            nc.vector.tensor_mul(
                                out=ex[:, :, d0:d0 + P],
                                in0=ex[:, :, d0:d0 + P], in1=tri_bc)
                        fillers.tick()
                        # AV for the previous kb-group: one group behind the
                        # score matmuls, so the PE never parks on exp(kb)
                        if pending_av is not None:
                            pending_av()
                        def make_av(kb=kb, s0=s0, ex=ex):
                            def go():
                                for hh in (0, 1):
                                    nc.tensor.matmul(
                                        avs[hh][:, s0:QS],
                                        lhsT=vst_tiles[kb][:, 2 * pr + hh, :],
                                        rhs=ex[:, hh, s0:QS],
                                        start=(kb == 0),
                                        stop=(kb == last_kb))
                            return go
                        pending_av = make_av()
                        pop_kb = 1 if last_pair else max(1, last_kb - 1)
                        if kb == pop_kb and pending_norm:
                            pending_norm.popleft()()
                        fillers.tick()
                    pending_av()
                    # drain av psum (GpSimd): raw attn rows + den rows
                    dr = drow_pool.tile([1, QS], BF16, name=f"dr_{pr}_{q4}",
                                        tag="drow")
                    tmp = tmp_pool.tile([HD + 1, QS], BF16,
                                        name=f"tmp_{pr}_{q4}", tag="tmp")
                    nc.vector.tensor_copy(
                        out=attn_sb[0:HD, pr, q0:q0 + QS], in_=avs[0][0:HD, :])
                    nc.vector.tensor_copy(out=dr, in_=avs[0][HD:HD + 1, :])
                    nc.scalar.copy(out=tmp, in_=avs[1])
                    fast_tail = last_pair and q4 == NQ - 1
                    if not fast_tail:
                        for hh, src in ((0, dr[0:1, :]),
                                        (1, tmp[HD:HD + 1, :])):
                            nc.sync.dma_start(
                                out=den[hh * RPH:(hh + 1) * RPH, q4, :],
                                in_=src)

                    def normalize(q4=q4, q0=q0, den=den, recl=recl, tmp=tmp,
                                  dr=dr, pr=pr, last_pair=last_pair,
                                  fast_tail=fast_tail):
                        if fast_tail:
                            # tail: reciprocal directly on the raw den rows
                            # (partitions 0 and 64) and two K=1 broadcasts —
                            # skips both DMA bounce trips
                            with nc.allow_low_precision(
                                    reason="bf16 softmax denominator"):
                                nc.vector.reciprocal(out=dr, in_=dr)
                                nc.vector.reciprocal(
                                    out=tmp[HD:HD + 1, :],
                                    in_=tmp[HD:HD + 1, :])
                            bps = av_ps.tile([P, QS], F32,
                                             name=f"bpst_{pr}_{q4}",
                                             tag="av")
                            nc.tensor.matmul(
                                bps[0:HD, :], lhsT=onep_sb[0:1, :], rhs=dr,
                                start=True, stop=True)
                            nc.tensor.matmul(
                                bps[HD:P, :], lhsT=onep_sb[HD:HD + 1, :],
                                rhs=tmp[HD:HD + 1, :],
                                start=True, stop=True)
                            sl = attn_sb[0:HD, pr, q0:q0 + QS]
                            nc.vector.tensor_mul(out=sl, in0=sl,
                                                 in1=bps[0:HD, :])
                            nc.vector.tensor_mul(out=tmp[0:HD, :],
                                                 in0=tmp[0:HD, :],
                                                 in1=bps[HD:P, :])
                            nc.sync.dma_start(
                                out=attn_sb[HD:P, pr, q0:q0 + QS],
                                in_=tmp[0:HD, :])
                            return
                        dsl = den[:, q4, :]
                        with nc.allow_low_precision(
                                reason="bf16 softmax denominator"):
                            nc.vector.reciprocal(out=dsl, in_=dsl)
                        for hh in (0, 1):
                            nc.sync.dma_start(
                                out=recl[hh:hh + 1, q4, :],
                                in_=den[hh * RPH:(hh + 1) * RPH, q4, :])
                        # one K=2 broadcast: rows 0:64 get head0's recip,
                        # rows 64:128 head1's (sel2 is the 0/1 selector)
                        bps = mm_ps.tile([P, QS], F32,
                                         name=f"bps_{pr}_{q4}", tag="mm")
                        nc.tensor.matmul(
                            bps, lhsT=sel2_sb, rhs=recl[:, q4, :],
                            start=True, stop=True)
                        sl = attn_sb[0:HD, pr, q0:q0 + QS]
                        nc.vector.tensor_mul(out=sl, in0=sl,
                                             in1=bps[0:HD, :])
                        nc.vector.tensor_mul(out=tmp[0:HD, :],
                                             in0=tmp[0:HD, :],
                                             in1=bps[HD:P, :])
                        nc.sync.dma_start(
                            out=attn_sb[HD:P, pr, q0:q0 + QS],
                            in_=tmp[0:HD, :])
                        if last_pair:
                            for qt in range(q4 * (QS // P),
                                            (q4 + 1) * (QS // P)):
                                for f0 in range(0, D, 512):
                                    for p_ in out_chain_parts(
                                            qt, f0, min(512, D - f0)):
                                        fillers.push(p_)
                            # pace the queue across ALL remaining ticks of
                            # this pair, so the final chunk (which has no
                            # other filler source) stays supplied and the PE
                            # holds its fast p-state to the end
                            rem = 2 * (QS // P) * (q4 + 2) - 2
                            for cc in range(q4 + 2, NQ):
                                rem += 2 * (QS // P) * (cc + 1)
                            fillers.set_rate(len(fillers.q) / max(rem, 1))

                    pending_norm.append(normalize)

            for pr in range(PAIRS):
                if pr == PAIRS - 1:
                    # switch pacing to out-proj chain slices (2/kb-group)
                    fillers.drain()
                    fillers.emitted = 0
                    fillers.ticks = 0
                    fillers.rate = 1.0
                attention_pair(pr, pr == PAIRS - 1)
            fillers.drain()
            # tail: the final chunk's out chains. Emit their j=0,1 halves
            # (pairs 0,1 — long since final) BEFORE the fast normalize so
            # they hide its latency; j=2,3 halves follow it.
            q4t = NQ - 1
            tail_specs = [(qt, f0)
                          for qt in range(q4t * (QS // P),
                                          (q4t + 1) * (QS // P))
                          for f0 in range(0, D, 512)]
            tail_parts = [
                out_chain_parts(qt, f0, min(512, D - f0),
                                pool=(st_ps if k % 2 else mm_ps),
                                ptag=("st" if k % 2 else "mm"))
                for k, (qt, f0) in enumerate(tail_specs)]
            for k in range(min(4, len(tail_parts))):
                tail_parts[k][0]()
            while pending_norm:
                pending_norm.popleft()()
            for k in range(len(tail_parts)):
                tail_parts[k][1]()
                if k + 4 < len(tail_parts):
                    tail_parts[k + 4][0]()

    nc.compile()
    return nc


def to_bf16(a):
    import ml_dtypes
    return np.ascontiguousarray(a).astype(ml_dtypes.bfloat16)


def make_core_inputs(x, Wqkv_w, Wqkv_b, out_w, out_b, H, n_tp):
    """Host-side shard + layout prep. Returns list of in_maps (one per core).
    Core c handles batch c // n_tp, head group c % n_tp."""
    B, L, D = x.shape
    hpg = H // n_tp            # heads per core
    PAIRS = hpg // 2
    EQK = 2 * hpg * HD
    EV = hpg * HD
    ET = EQK // P
    tri = np.triu(np.ones((P, P), dtype=np.float32))  # [k, q]: 1 if q >= k
    in_maps = []
    for c in range(B * n_tp):
        b, g = c // n_tp, c % n_tp
        # qk row order: per pair p -> q(2p), q(2p+1), k(2p), k(2p+1)
        qk_rows = []
        for p_ in range(PAIRS):
            for h in (2 * p_, 2 * p_ + 1):
                qk_rows.extend(range(g * hpg * HD + h * HD,
                                     g * hpg * HD + h * HD + HD))
            for h in (2 * p_, 2 * p_ + 1):
                qk_rows.extend(range(D + g * hpg * HD + h * HD,
                                     D + g * hpg * HD + h * HD + HD))
        v_rows = list(range(2 * D + g * hpg * HD, 2 * D + (g + 1) * hpg * HD))
        rows = np.array(qk_rows + v_rows)
        in_maps.append({
            "xT": to_bf16(x[b].T),
            "wT": to_bf16(Wqkv_w[rows].T),
            "bqk": np.ascontiguousarray(
                Wqkv_b[np.array(qk_rows)].reshape(ET, P).T),
            "vb": np.tile(Wqkv_b[np.array(v_rows)], (P, 1)),
            "woT": to_bf16(out_w[:, g * EV:(g + 1) * EV].T),
            "ob": (np.tile(out_b, (P, 1)) if g == 0
                   else np.zeros((P, D), np.float32)),
            "tri": to_bf16(tri),
            "onep": to_bf16(np.ones((P, HD), np.float32)),
            "sel2": to_bf16(np.concatenate([
                np.concatenate([np.ones((1, HD)), np.zeros((1, HD))], 1),
                np.concatenate([np.zeros((1, HD)), np.ones((1, HD))], 1),
            ]).astype(np.float32)),
            "onec": to_bf16(np.ones((P, L // P), np.float32)),
        })
    return in_maps


_NC_CACHE = {}
LAST_RESULTS = None


def kernel(x, Wqkv_w, Wqkv_b, out_w, out_b):
    global LAST_RESULTS
    x = np.asarray(x, dtype=np.float32)
    Wqkv_w = np.asarray(Wqkv_w, dtype=np.float32)
    Wqkv_b = np.asarray(Wqkv_b, dtype=np.float32)
    out_w = np.asarray(out_w, dtype=np.float32)
    out_b = np.asarray(out_b, dtype=np.float32)

    B, L, D = x.shape
    H = 16
    n_tp = 2
    hpg = H // n_tp

    key = (L, D, hpg)
    if key not in _NC_CACHE:
        _NC_CACHE[key] = build_mha_nc(L, D, hpg)
    nc = _NC_CACHE[key]

    in_maps = make_core_inputs(x, Wqkv_w, Wqkv_b, out_w, out_b, H, n_tp)

    from concourse.bass_utils import run_bass_kernel_spmd
    res = run_bass_kernel_spmd(nc, in_maps, core_ids=list(range(len(in_maps))))
    LAST_RESULTS = res

    out = np.empty((B, L, D), dtype=np.float32)
    for b in range(B):
        out[b] = res.results[n_tp * b]["out"]
        for g in range(1, n_tp):
            out[b] += res.results[n_tp * b + g]["out"]
    return out


if __name__ == "__main__":
    nc = build_mha_nc(2048, 1024, 8)
    print("built OK")
